# revision 53
# baseline (speedup 1.0000x reference)
"""Trainium2 Bass kernel for a Gaussian-splat rendering loss.

Full inputs -> scalar loss. Sharding: 8 cores = 2 batches x 4 row-bands.
Each core renders a 38-row window (32 owned rows + 3-row halo each side)
of one batch image against all 1024 depth-sorted gaussians, computes its
partial loss sums on-device, and the host combines 8 partial vectors.

Per-gaussian preprocessing (projection, EWA 2D covariance, colors) runs
on the host (tiny: O(N) numpy) so each core only receives the compact
derived tensors; the pixel-monomial matrix Phi is reconstructed on device
from a small basis (rank-3 factorization) instead of being shipped.

Device algorithm (per core):
  - splat power computed as a PE matmul: power[pix,n] = Phi[pix,:] @ Psi[:,n]
    where Phi are pixel monomials [gx^2, gx*gy, gy^2, gx, gy, 1] (centered)
  - alpha/transmittance compositing via a multiplicative scan over sorted
    gaussians; weighted color/depth reduction via Abel summation
    (sum_n w[n]*col[n] = col[0] + sum_n c[n]*(col[n+1]-col[n]))
  - separable 7x7 gaussian SSIM on the rendered window
  - L1 / SSIM / depth / opacity-entropy partial sums -> [6] outputs
"""

import os
import numpy as np

B, N, H, W = 2, 1024, 128, 128
R = 38          # window rows per core (32 owned + 3 halo each side)
RP = 40         # R padded to a multiple of 4 for the hardware loop
OWN = 32
NCORES = 8
C0 = 0.28209479177387814
C1 = 0.01 ** 2
C2 = 0.03 ** 2
EXP_N10 = float(np.exp(np.float32(-10.0)))  # exp(-10) in f32

NPIX_RGB = float(B * 3 * H * W)
NPIX_D = float(B * 1 * H * W)
NGAUSS = float(B * N)

PF_LEN = 6 * N + N + 4     # Psi + negop + c0z0, f32 words
PF_SH = PF_LEN // 4        # 1793 per-core shard
PH_LEN = 128 * 8 * 4       # dczT, bf16 words
PH_SH = PH_LEN // 4        # 1024 per-core shard


def _ssim_g7():
    coords = np.arange(7, dtype=np.float32) - 3
    g = np.exp(-coords ** 2 / (2 * np.float32(1.5) ** 2))
    g = g / g.sum()
    return g.astype(np.float32)

G7 = _ssim_g7()


# --------------------------------------------------------------------------
# host-side per-gaussian preprocessing (numpy f32, O(N) - not the hot path)
# --------------------------------------------------------------------------

def _prep_gaussians_np(g, intr):
    """Projection + EWA 2D covariance + colors, all in f32."""
    f = np.float32
    g = g.astype(f)
    x, y, z3 = g[:, 0], g[:, 1], g[:, 2]
    s0, s1, s2 = g[:, 3], g[:, 4], g[:, 5]
    qw, qx, qy, qz = g[:, 6], g[:, 7], g[:, 8], g[:, 9]
    opac = g[:, 10]
    intr = intr.reshape(9)
    fx, cx, fy, cy = intr[0], intr[2], intr[4], intr[5]

    zcl = np.maximum(z3, f(1e-4))
    rz = f(1.0) / zcl
    px = (x * rz) * fx + cx
    py = (y * rz) * fy + cy
    zc6 = np.maximum(z3, f(1e-6))
    rzc = f(1.0) / zc6
    aJ = rzc * fx
    cJ = rzc * fy
    rzsq = rzc * rzc
    bJ = (x * rzsq) * (-fx)
    dJ = (y * rzsq) * (-fy)

    xx, yy, zz = qx * qx, qy * qy, qz * qz
    xy, xz, yz = qx * qy, qx * qz, qy * qz
    wx, wy, wz = qw * qx, qw * qy, qw * qz
    r00 = (yy + zz) * f(-2) + f(1)
    r01 = (xy - wz) * f(2)
    r02 = (xz + wy) * f(2)
    r10 = (xy + wz) * f(2)
    r11 = (xx + zz) * f(-2) + f(1)
    r12 = (yz - wx) * f(2)
    r20 = (xz - wy) * f(2)
    r21 = (yz + wx) * f(2)
    r22 = (xx + yy) * f(-2) + f(1)
    ss0, ss1, ss2 = s0 * s0, s1 * s1, s2 * s2
    t00, t01, t02 = r00 * ss0, r01 * ss1, r02 * ss2
    t10, t11, t12 = r10 * ss0, r11 * ss1, r12 * ss2
    t20, t21, t22 = r20 * ss0, r21 * ss1, r22 * ss2
    Ca = (r00 * t00 + r01 * t01) + r02 * t02
    Cb = (r00 * t10 + r01 * t11) + r02 * t12
    Cc = (r00 * t20 + r01 * t21) + r02 * t22
    Cd = (r10 * t10 + r11 * t11) + r12 * t12
    Ce = (r10 * t20 + r11 * t21) + r12 * t22
    Cf = (r20 * t20 + r21 * t21) + r22 * t22

    a2, ab, b2 = aJ * aJ, aJ * bJ, bJ * bJ
    c2, cd, d2 = cJ * cJ, cJ * dJ, dJ * dJ
    c00 = (a2 * Ca + b2 * Cf) + (ab * Cc) * f(2) + f(0.3)
    c11 = (c2 * Cd + d2 * Cf) + (cd * Ce) * f(2) + f(0.3)
    ac, ad, bc, bd = aJ * cJ, aJ * dJ, bJ * cJ, bJ * dJ
    c01 = (ac * Cb + ad * Cc) + (bc * Ce + bd * Cf)
    det = np.maximum(c00 * c11 - c01 * c01, f(1e-8))
    rdet = f(1.0) / det
    i00 = c11 * rdet
    i11 = c00 * rdet
    ni01 = c01 * rdet  # = -inv01

    pxc = px - f(64)
    pyc = py - f(64)
    psi = np.zeros((6, N), np.float32)
    psi[0] = i00 * f(-0.5)
    psi[1] = ni01
    psi[2] = i11 * f(-0.5)
    psi[3] = i00 * pxc - ni01 * pyc
    psi[4] = i11 * pyc - ni01 * pxc
    psi[5] = (pxc * psi[3] + pyc * psi[4]) * f(-0.5)

    col = np.clip(g[:, 11:14] * f(C0) + f(0.5), 0.0, 1.0).astype(f)  # [N,3]
    colz = np.concatenate([col, zcl[:, None]], axis=1)  # [N,4]
    dcolz = np.empty_like(colz)
    dcolz[:-1] = colz[1:] - colz[:-1]
    dcolz[-1] = -colz[-1]
    return psi, colz, dcolz, opac


# --------------------------------------------------------------------------
# host-side sharding
# --------------------------------------------------------------------------

def shard_inputs(gaussians, intrinsics, target_rgb, target_depth):
    gaussians = np.ascontiguousarray(gaussians, dtype=np.float32)
    intrinsics = np.ascontiguousarray(intrinsics, dtype=np.float32)
    target_rgb = np.ascontiguousarray(target_rgb, dtype=np.float32)
    target_depth = np.ascontiguousarray(target_depth, dtype=np.float32)

    z = np.maximum(gaussians[:, :, 2], 1e-4)
    order = np.argsort(z, axis=1, kind="stable")
    gs = np.take_along_axis(gaussians, order[:, :, None], axis=1)  # [B,N,38]

    # per-batch derived tensors
    per_b = []
    import ml_dtypes
    for b in range(B):
        psi, colz, dcolz, opac = _prep_gaussians_np(gs[b], intrinsics[b])
        dczT = np.ascontiguousarray(
            dcolz.reshape(8, 128, 4).transpose(1, 0, 2)
        ).astype(ml_dtypes.bfloat16)  # [128(p),8(f),4], n = f*128+p
        c0z0 = np.ascontiguousarray(colz[0].reshape(4, 1))
        negop = np.ascontiguousarray((-opac).reshape(1, N))
        # batch-replicated data is uploaded as 4 shards (one per core of the
        # batch group) and AllGathered on device: f32 pack + bf16 pack
        pack_f = np.concatenate([psi.ravel(), negop.ravel(),
                                 c0z0.ravel()]).astype(np.float32)  # [7172]
        pack_h = dczT.ravel()  # [4096] bf16
        per_b.append((pack_f.reshape(4, PF_SH), pack_h.reshape(4, PH_SH)))

    gx = np.arange(W, dtype=np.float32) - 64.0
    basis = np.stack([gx * gx, gx, np.ones_like(gx)])  # [3, W]

    in_maps = []
    for c in range(NCORES):
        b, q = divmod(c, 4)
        pack_f, pack_h = per_b[b]
        row0 = q * OWN
        wr = np.arange(row0 - 3, row0 + OWN + 3)
        valid = (wr >= 0) & (wr < H)
        wrc = np.clip(wr, 0, H - 1)
        gyv = np.where(valid, wr.astype(np.float32) - 64.0, 0.0).astype(np.float32)

        # rank-3 factorization of Phi: phi[k,r,w] = sum_c basis[c,w]*CT[c,6r+k]
        # (rows R..RP-1 are zero padding for the hardware loop; their rendered
        # output is never read)
        CT = np.zeros((3, RP * 6), np.float32)
        for r in range(R):
            gy = gyv[r]
            CT[0, 6 * r + 0] = 1.0              # gx^2
            CT[1, 6 * r + 1] = gy               # gy*gx
            CT[2, 6 * r + 2] = gy * gy          # gy^2
            CT[1, 6 * r + 3] = 1.0              # gx
            CT[2, 6 * r + 4] = gy               # gy
            CT[2, 6 * r + 5] = 1.0              # 1
        ctb = np.ascontiguousarray(np.concatenate([CT, basis], axis=1))  # [3, 228+128]

        targ4 = np.zeros((4, R, W), np.float32)
        targ4[0:3, valid, :] = target_rgb[b][:, wrc[valid], :]
        targ4[3, valid, :] = target_depth[b, 0, wrc[valid], :]

        rowmask = np.zeros((128, 1), np.float32)
        rowmask[: 3 * R, 0] = np.tile(valid.astype(np.float32), 3)

        in_maps.append({
            "repf": np.ascontiguousarray(pack_f[q].reshape(1, PF_SH)),
            "reph": np.ascontiguousarray(pack_h[q].reshape(1, PH_SH)),
            "ctb": ctb,
            "targ4": targ4.astype(ml_dtypes.bfloat16),
            "rowmask": rowmask,
        })
    return in_maps


def entropy_host(gaussians):
    """Opacity-entropy term: depends only on the raw inputs, so it is
    computed on the host (the sorted order does not change the sum)."""
    f = np.float32
    o = np.clip(gaussians[:, :, 10].astype(f), f(1e-6), f(1.0 - 1e-6))
    return float((o * np.log(o) + (1 - o) * np.log(1 - o)).sum(dtype=np.float64))


def combine(partials_list, ent_sum):
    S = np.zeros(5, np.float64)
    for p in partials_list:
        S += p[:5].astype(np.float64)
    l1_rgb = (S[0] + S[1] + S[2]) / NPIX_RGB
    l1_depth = S[3] / NPIX_D
    ssim = S[4] / NPIX_RGB
    ent = -ent_sum / NGAUSS
    loss = 0.8 * l1_rgb + 0.2 * (1.0 - ssim) + 0.5 * l1_depth + 0.01 * ent
    return np.float32(loss)


# --------------------------------------------------------------------------
# numpy mirror of the device program (for algorithm validation)
# --------------------------------------------------------------------------

def _conv7_np(x, axis):
    """SAME zero-padded 7-tap conv along given axis, f32, mirrors device order."""
    out = np.zeros_like(x)
    n = x.shape[axis]
    xm = np.moveaxis(x, axis, 0)
    om = np.moveaxis(out, axis, 0)
    om[:] = xm * G7[3]
    for k in [0, 1, 2, 4, 5, 6]:
        lo = max(0, 3 - k)
        hi = n + min(0, 3 - k)
        om[lo:hi] += xm[lo + k - 3: hi + k - 3] * G7[k]
    return out


def mirror_core(m, pack_f, pack_h):
    """Numpy mirror of one core's device program. Returns partials [6]."""
    f = np.float32
    psi = pack_f[0:6 * N].reshape(6, N)
    negop = pack_f[6 * N:7 * N]
    c0z0 = pack_f[7 * N:7 * N + 4]
    dcolz = pack_h.astype(f).reshape(128, 8, 4).transpose(1, 0, 2).reshape(N, 4)
    CT = m["ctb"][:, :RP * 6]
    basis = m["ctb"][:, RP * 6:]

    # render
    rend = np.zeros((4, R, W), np.float32)
    for r in range(R):
        phi_r = (CT[:, 6 * r:6 * r + 6].T @ basis).astype(f)    # [6, W]
        power = (phi_r.T.astype(f) @ psi.astype(f)).astype(f)   # [W, N]
        e = np.exp(power).astype(f)
        mneg = np.maximum(e, f(EXP_N10)) * negop[None, :]
        oma = np.maximum(mneg + f(1.0), f(0.01)).astype(f)
        c = np.cumprod(oma, axis=1, dtype=f)  # [W, N]
        acc = (c @ dcolz.astype(f)).astype(f)  # [W, 4]
        rend[0:3, r, :] = np.maximum(acc[:, 0:3] + c0z0[0:3], f(0.0)).T
        rend[3, r, :] = acc[:, 3] + c0z0[3]
    rend[0:3] = np.minimum(rend[0:3], f(1.0))

    # l1 losses (owned rows only)
    targ4 = m["targ4"].astype(f)
    omask = np.zeros((4, R, W), np.float32)
    omask[:, 3:3 + OWN, :] = 1.0
    l1d = np.abs(rend - targ4).astype(f)
    lacc = (l1d * omask).reshape(4, -1).sum(axis=1, dtype=f)

    # ssim on the window
    rowmask = m["rowmask"][: 3 * R, 0].reshape(3, R)
    img1 = rend[0:3] * rowmask[:, :, None]
    img2 = targ4[0:3]
    i11 = img1 * img1
    i22 = img2 * img2
    i12 = img1 * img2
    outs = []
    for xin in (img1, img2, i11, i22, i12):
        rc = _conv7_np(xin.astype(f), axis=2)     # along W
        hc = _conv7_np(rc.astype(f), axis=1)      # along rows (full window)
        outs.append(hc[:, 3:3 + OWN, :].astype(f))
    mu1, mu2, M11, M22, M12 = outs
    A = mu1 * mu2
    num = (A * f(2) + f(C1)) * ((M12 - A) * f(2) + f(C2))
    Cq = mu1 * mu1
    Dq = mu2 * mu2
    den = ((Cq + f(C1)) + Dq) * (((M11 - Cq) + f(C2)) + (M22 - Dq))
    smap = (num / den).astype(f)
    ssum = smap.sum(dtype=f)

    return np.array([lacc[0], lacc[1], lacc[2], lacc[3], ssum, 0.0], np.float32)


def kernel_numpy(**inputs):
    """Full numpy mirror (no device) - for validation."""
    in_maps = shard_inputs(**inputs)
    partials = []
    for c, m in enumerate(in_maps):
        g = (c // 4) * 4
        pack_f = np.concatenate([in_maps[g + q]["repf"][0] for q in range(4)])
        pack_h = np.concatenate([in_maps[g + q]["reph"][0] for q in range(4)])
        partials.append(mirror_core(m, pack_f, pack_h))
    return combine(partials, entropy_host(np.asarray(inputs["gaussians"],
                                                     dtype=np.float32)))


# --------------------------------------------------------------------------
# device program
# --------------------------------------------------------------------------

_PROG_CACHE = {}


def build_program(debug_rend=False):
    import concourse.bass as bass
    import concourse.bacc as bacc
    import concourse.tile as tile
    import concourse.mybir as mybir
    from concourse.masks import make_identity

    F32 = mybir.dt.float32
    BF16 = mybir.dt.bfloat16
    OP = mybir.AluOpType
    ACT = mybir.ActivationFunctionType

    nc = bacc.Bacc("TRN2", target_bir_lowering=False, debug=False,
                   num_devices=NCORES)
    repf_in = nc.dram_tensor("repf", [1, PF_SH], F32, kind="ExternalInput").ap()
    reph_in = nc.dram_tensor("reph", [1, PH_SH], BF16, kind="ExternalInput").ap()
    ctb_in = nc.dram_tensor("ctb", [3, RP * 6 + W], F32, kind="ExternalInput").ap()
    targ4_in = nc.dram_tensor("targ4", [4, R, W], BF16, kind="ExternalInput").ap()
    rowmask_in = nc.dram_tensor("rowmask", [128, 1], F32, kind="ExternalInput").ap()
    partials = nc.dram_tensor("partials", [6], F32, kind="ExternalOutput").ap()
    if debug_rend:
        dbg_rend = nc.dram_tensor("dbg_rend", [4, R, W], F32, kind="ExternalOutput").ap()

    V = nc.vector
    S = nc.scalar
    T = nc.tensor
    G = nc.gpsimd

    with tile.TileContext(nc) as tc:
        with (
            tc.tile_pool(name="const", bufs=1) as cp,
            tc.tile_pool(name="loop", bufs=1) as lp,
            tc.tile_pool(name="ppow", bufs=1, space="PSUM") as ppow,
            tc.tile_pool(name="pmisc", bufs=1, space="PSUM") as pmisc,
            tc.tile_pool(name="dram", bufs=1, space="DRAM") as dp,
        ):
            # ---------------- constants / loads ----------------
            idt = cp.tile([128, 128], F32, tag="identity", name="identity")
            make_identity(nc, idt[:])
            ones_col = cp.tile([128, 1], F32, tag="ones_col", name="ones_col")
            G.memset(ones_col[:], 1.0)
            ones_row = cp.tile([1, 128], F32, tag="ones_row", name="ones_row")
            G.memset(ones_row[:], 1.0)

            # gather the batch-replicated packs from the 4 cores of the group
            agf_in = dp.tile([1, PF_SH], F32, tag="agf_in", name="agf_in")
            nc.sync.dma_start(agf_in[:], repf_in[:])
            agf = dp.tile([1, PF_LEN], F32, tag="agf", name="agf")
            G.collective_compute(
                "AllGather", mybir.AluOpType.bypass,
                replica_groups=[[0, 1, 2, 3], [4, 5, 6, 7]],
                ins=[agf_in[:]], outs=[agf[:]])
            agh_in = dp.tile([1, PH_SH], BF16, tag="agh_in", name="agh_in")
            nc.sync.dma_start(agh_in[:], reph_in[:])
            agh = dp.tile([1, PH_LEN], BF16, tag="agh", name="agh")
            G.collective_compute(
                "AllGather", mybir.AluOpType.bypass,
                replica_groups=[[0, 1, 2, 3], [4, 5, 6, 7]],
                ins=[agh_in[:]], outs=[agh[:]])

            Psi = cp.tile([6, N], F32, tag="Psi", name="Psi")
            nc.sync.dma_start(Psi[:],
                              agf[0, 0:6 * N].rearrange("(k n) -> k n", k=6))
            nrow = cp.tile([1, N], F32, tag="nrow", name="nrow")
            nc.sync.dma_start(nrow[:],
                              agf[0, 6 * N:7 * N].rearrange("(a n) -> a n", a=1))
            c0z0 = cp.tile([4, 1], F32, tag="c0z0", name="c0z0")
            nc.sync.dma_start(c0z0[:],
                              agf[0, 7 * N:7 * N + 4].rearrange("(a b) -> a b", a=4))
            dcz = cp.tile([128, 8, 4], BF16, tag="dcz", name="dcz")
            nc.sync.dma_start(dcz[:],
                              agh[0, :].rearrange("(p k c) -> p k c", p=128, k=8))
            ctb = cp.tile([3, RP * 6 + W], F32, tag="ctb", name="ctb")
            nc.sync.dma_start(ctb[:], ctb_in[:])

            targ4_bf = cp.tile([4, R, W], BF16, tag="targ4_bf", name="targ4_bf")
            nc.sync.dma_start(targ4_bf[:], targ4_in[:])
            targ4_sb = cp.tile([4, R, W], F32, tag="targ4_sb", name="targ4_sb")
            V.tensor_copy(targ4_sb[:], targ4_bf[:])
            targc_bf = cp.tile([128, W], BF16, tag="targc_bf", name="targc_bf")
            G.memset(targc_bf[:], 0.0)
            nc.sync.dma_start(targc_bf[0:114, :],
                              targ4_in[0:3].rearrange("c r w -> (c r) w"))
            targc = cp.tile([128, W], F32, tag="targc", name="targc")
            V.tensor_copy(targc[:], targc_bf[:])
            rowm = cp.tile([128, 1], F32, tag="rowm", name="rowm")
            nc.sync.dma_start(rowm[:], rowmask_in[:])

            omask = cp.tile([4, R, W], F32, tag="omask", name="omask")
            G.memset(omask[:], 0.0)
            G.memset(omask[:, 3:3 + OWN, :], 1.0)

            # ---------------- negop broadcast [128, N] ----------------
            negopb = cp.tile([128, N], F32, tag="negopb", name="negopb")
            nps = ppow.tile([128, N], F32, tag="pow0", name="nps")
            T.matmul(nps[:, 0:512], ones_row[:], nrow[:, 0:512], start=True, stop=True)
            T.matmul(nps[:, 512:1024], ones_row[:], nrow[:, 512:1024], start=True, stop=True)
            V.tensor_copy(negopb[:], nps[:])

            # ---------------- Phi reconstruction ----------------
            # phiT[w, 6r+k] = sum_c basis[c,w] * CT[c,6r+k]; PE-transpose to
            # [(6r+k), w] chunks, bounce through DRAM; the render loop stages
            # each row's [6, W] slice back by DMA (PE lhsT needs a static
            # address, and base partition must be 0/32/64).
            KR = RP * 6  # 240
            bphi = pmisc.tile([128, KR], F32, tag="tp", name="bphi")
            T.matmul(bphi[:], ctb[:, KR:KR + W], ctb[:, 0:KR], start=True, stop=True)
            phiT_sb = cp.tile([128, KR], F32, tag="phiT_sb", name="phiT_sb")
            V.tensor_copy(phiT_sb[:], bphi[:])
            phiP = cp.tile([128, 2, 128], F32, tag="phiP", name="phiP")
            trA = pmisc.tile([128, 128], F32, tag="tp", name="trA")
            T.transpose(trA[0:126, :], phiT_sb[:, 0:126], idt[:])
            V.tensor_copy(phiP[0:126, 0, :], trA[0:126, :])
            trB = pmisc.tile([128, 128], F32, tag="tp", name="trB")
            T.transpose(trB[0:114, :], phiT_sb[:, 126:240], idt[:])
            V.tensor_copy(phiP[0:114, 1, :], trB[0:114, :])
            phi_scr = dp.tile([KR, W], F32, tag="phi_scr", name="phi_scr")
            nc.sync.dma_start(phi_scr[0:126, :], phiP[0:126, 0, :])
            nc.sync.dma_start(phi_scr[126:240, :], phiP[0:114, 1, :])

            # ---------------- render loop (hardware loop, 4 rows/iter) ------
            rend_all = cp.tile([4, RP, W], F32, tag="rend_all", name="rend_all")
            NK = N // 128
            rend_flat = rend_all[:].rearrange("c r w -> c (r w)")

            # statically allocated per-slot tiles (manual double buffer)
            pw_t = [ppow.tile([128, N], F32, tag=f"pow{j}", name=f"pow{j}")
                    for j in range(2)]
            er_t = [lp.tile([128, N], F32, tag=f"eraw{j}", name=f"eraw{j}")
                    for j in range(2)]
            om_t = [lp.tile([128, N], F32, tag=f"oma{j}", name=f"oma{j}")
                    for j in range(2)]
            ct_t = [lp.tile([128, N], BF16, tag=f"ctile{j}", name=f"ctile{j}")
                    for j in range(2)]
            csb_t = [lp.tile([128, NK, 128], BF16, tag=f"ctsb{j}", name=f"ctsb{j}")
                     for j in range(2)]
            phi_t = [lp.tile([6, W], F32, tag=f"phis{j}", name=f"phis{j}")
                     for j in range(2)]
            acc2 = pmisc.tile([4, 2, 128], F32, tag="acc", name="acc")

            from concourse.bass import ds

            def render_row(row_i, j):
                """row_i: ScalarValue row index; j: buffer slot (0/1)."""
                pw, er, om, ct = pw_t[j], er_t[j], om_t[j], ct_t[j]
                csb, acc = csb_t[j], acc2[:, j, :]
                phi_r = phi_t[j]
                nc.sync.dma_start(phi_r[:], phi_scr[ds(row_i * 6, 6), :])
                T.matmul(pw[:, 0:512], phi_r[:], Psi[:, 0:512],
                         start=True, stop=True)
                T.matmul(pw[:, 512:1024], phi_r[:], Psi[:, 512:1024],
                         start=True, stop=True)
                S.activation(er[:], pw[:], ACT.Exp, bias=0.0, scale=1.0)
                V.scalar_tensor_tensor(om[:], er[:], EXP_N10, negopb[:],
                                       OP.max, OP.mult)
                G.tensor_scalar(om[:], om[:], 1.0, 0.01, OP.add, OP.max)
                V.tensor_tensor_scan(ct[:], om[:], om[:], 1.0, OP.mult, OP.bypass)
                nc.sync.dma_start_transpose(csb[:], ct[:])
                for k in range(NK):
                    T.matmul(acc, dcz[:, k, :], csb[:, k, :],
                             start=(k == 0), stop=(k == NK - 1))
                V.tensor_scalar(rend_flat[:, ds(row_i * W, W)], acc,
                                c0z0[:, :], 0.0, OP.add, OP.max)

            with tc.For_i(0, RP, 4, staggered_reset=True) as rv:
                for jj in range(4):
                    render_row(rv + jj, jj % 2)

            V.tensor_scalar(rend_all[0:3], rend_all[0:3], 1.0, None,
                            OP.min, OP.bypass)

            if debug_rend:
                nc.sync.dma_start(dbg_rend[:], rend_all[:, 0:R, :])

            # ---------------- L1 losses ----------------
            l1d = cp.tile([4, R, W], F32, tag="l1d", name="l1d")
            V.tensor_sub(l1d[:], rend_all[:, 0:R, :], targ4_sb[:])
            S.activation(l1d[:], l1d[:], ACT.Abs, bias=0.0, scale=1.0)
            lacc = cp.tile([4, 1], F32, tag="lacc", name="lacc")
            V.tensor_mul(l1d[:], l1d[:], omask[:])
            V.tensor_reduce(lacc[:], l1d[:], axis=mybir.AxisListType.XY, op=OP.add)

            # ---------------- SSIM ----------------
            img1 = cp.tile([128, W], F32, tag="img1", name="img1")
            G.memset(img1[:], 0.0)
            for ch in range(3):
                nc.sync.dma_start(img1[ch * R:(ch + 1) * R, :],
                                  rend_all[ch:ch + 1, 0:R, :])
            V.tensor_scalar(img1[:], img1[:], rowm[:], None,
                            OP.mult, OP.bypass)
            i11t = cp.tile([128, W], F32, tag="i11t", name="i11t")
            V.tensor_mul(i11t[:], img1[:], img1[:])
            i22t = cp.tile([128, W], F32, tag="i22t", name="i22t")
            V.tensor_mul(i22t[:], targc[:], targc[:])
            i12t = cp.tile([128, W], F32, tag="i12t", name="i12t")
            V.tensor_mul(i12t[:], img1[:], targc[:])

            # separable 7-tap convs as banded-matrix matmuls.
            # Kw[p,f] = g7[f-p+3] (full-width band); Kb = same band restricted
            # to the three 38-row channel blocks (rows 0:114).
            g7 = [float(v) for v in G7]
            Kw = cp.tile([128, 128], F32, tag="Kw", name="Kw")
            G.memset(Kw[:], 0.0)
            for d in range(-3, 4):
                G.affine_select(out=Kw[:], in_=Kw[:],
                                compare_op=OP.not_equal, fill=g7[d + 3],
                                base=-d, pattern=[[1, 128]],
                                channel_multiplier=-1)
            # cross-channel leakage from the full-width row band only lands in
            # halo output rows (discarded by the 3:35 owned slice), and input
            # rows >= 114 are zero, so the same Kw serves both conv directions.
            convs = []
            for j, xin in enumerate([img1, targc, i11t, i22t, i12t]):
                p1 = pmisc.tile([128, 128], F32, tag="tp", name=f"cv{j}a")
                T.matmul(p1[0:114, :], Kw[:, 0:114], xin[:], start=True, stop=True)
                s1 = cp.tile([128, W], F32, tag=f"rc{j}", name=f"rc{j}")
                V.tensor_copy(s1[0:114, :], p1[0:114, :])
                p2 = pmisc.tile([128, 128], F32, tag="tp", name=f"cv{j}b")
                T.transpose(p2[:, 0:114], s1[0:114, :], idt[0:114, 0:114])
                s2 = cp.tile([128, 128], F32, tag=f"rcT{j}", name=f"rcT{j}")
                V.tensor_copy(s2[:, 0:114], p2[:, 0:114])
                s2v = s2[:, 0:114].rearrange("p (c r) -> p c r", c=3)
                p3 = pmisc.tile([128, 3, OWN], F32, tag="tp2", name=f"cv{j}c")
                T.matmul(p3[:], Kw[:], s2v[:, :, 3:3 + OWN], start=True, stop=True)
                mu = cp.tile([128, 3, OWN], F32, tag=f"mu{j}", name=f"mu{j}")
                V.tensor_copy(mu[:], p3[:])
                convs.append(mu)
            mu1, mu2, M11, M22, M12 = convs

            def big(tag):
                return cp.tile([128, 3, OWN], F32, tag=tag, name=tag)

            A = big("ssA")
            V.tensor_mul(A[:], mu1[:], mu2[:])
            num1 = big("ssnum1")
            V.tensor_scalar(num1[:], A[:], 2.0, C1, OP.mult, OP.add)
            Bv = big("ssB")
            V.tensor_sub(Bv[:], M12[:], A[:])
            num2 = big("ssnum2")
            V.tensor_scalar(num2[:], Bv[:], 2.0, C2, OP.mult, OP.add)
            num = big("ssnum")
            V.tensor_mul(num[:], num1[:], num2[:])
            Cq = big("ssC")
            V.tensor_mul(Cq[:], mu1[:], mu1[:])
            Dq = big("ssD")
            V.tensor_mul(Dq[:], mu2[:], mu2[:])
            den1 = big("ssden1")
            V.scalar_tensor_tensor(den1[:], Cq[:], C1, Dq[:], OP.add, OP.add)
            Ev = big("ssE")
            V.tensor_sub(Ev[:], M11[:], Cq[:])
            Fv = big("ssF")
            V.tensor_sub(Fv[:], M22[:], Dq[:])
            den2 = big("ssden2")
            V.scalar_tensor_tensor(den2[:], Ev[:], C2, Fv[:], OP.add, OP.add)
            den = big("ssden")
            V.tensor_mul(den[:], den1[:], den2[:])
            rden = big("ssrden")
            V.reciprocal(rden[:], den[:])
            smap = big("ssmap")
            V.tensor_mul(smap[:], num[:], rden[:])
            ssum = cp.tile([128, 1], F32, tag="ssum", name="ssum")
            V.tensor_reduce(ssum[:], smap[:], axis=mybir.AxisListType.XY, op=OP.add)
            sp = pmisc.tile([1, 1], F32, tag="tp", name="tp2")
            T.matmul(sp[:], ssum[:], ones_col[:], start=True, stop=True)

            # ---------------- outputs ----------------
            outsb = cp.tile([1, 1], F32, tag="outsb", name="outsb")
            V.tensor_copy(outsb[:, 0:1], sp[:])
            nc.sync.dma_start(partials[0:4], lacc[:, 0])
            nc.sync.dma_start(partials[4:5], outsb[0, :])

    nc.compile()
    return nc


def _get_program(debug_rend=False):
    key = ("prog", debug_rend)
    if key not in _PROG_CACHE:
        _PROG_CACHE[key] = build_program(debug_rend)
    return _PROG_CACHE[key]


def run_device(in_maps, mode="hw", debug_rend=False):
    nc = _get_program(debug_rend)
    if mode == "sim":
        from concourse.bass_interp import MultiCoreSim
        sim = MultiCoreSim(nc, num_cores=len(in_maps))
        for i, m in enumerate(in_maps):
            for k, v in m.items():
                sim.cores[i].tensor(k)[:] = v
        sim.simulate(check_with_hw=False)
        names = ["partials"] + (["dbg_rend"] if debug_rend else [])
        return [{n: np.array(sim.cores[i].tensor(n)) for n in names}
                for i in range(len(in_maps))]
    from concourse.bass_utils import run_bass_kernel_spmd
    res = run_bass_kernel_spmd(nc, in_maps, list(range(len(in_maps))))
    return res.results


def kernel(**inputs):
    in_maps = shard_inputs(**inputs)
    mode = os.environ.get("GK_MODE", "hw")
    results = run_device(in_maps, mode=mode)
    ent = entropy_host(np.asarray(inputs["gaussians"], dtype=np.float32))
    return combine([r["partials"] for r in results], ent)


if __name__ == "__main__":
    import jax
    with jax.default_device(jax.devices("cpu")[0]):
        import reference
        inputs = {k: np.asarray(v) for k, v in reference.setup_inputs().items()}
        expected = float(reference.reference(**inputs))
    got = float(kernel_numpy(**inputs))
    rel = abs(got - expected) / max(abs(expected), 1e-12)
    print(f"expected {expected:.8f}  mirror {got:.8f}  rel {rel:.3e}")


# revision 62
# speedup vs baseline: 5.9386x; 5.9386x over previous
"""Trainium2 Bass kernel for a Gaussian-splat rendering loss.

Full inputs -> scalar loss. Sharding: 8 cores = 2 batches x 4 row-bands.
Each core renders a 38-row window (32 owned rows + 3-row halo each side)
of one batch image against all 1024 depth-sorted gaussians, computes its
partial loss sums on-device, and the host combines 8 partial vectors.

Per-gaussian preprocessing (projection, EWA 2D covariance, colors) runs
on the host (tiny: O(N) numpy) so each core only receives the compact
derived tensors; the pixel-monomial matrix Phi is reconstructed on device
from a small basis (rank-3 factorization) instead of being shipped.

Device algorithm (per core):
  - splat power computed as a PE matmul: power[pix,n] = Phi[pix,:] @ Psi[:,n]
    where Phi are pixel monomials [gx^2, gx*gy, gy^2, gx, gy, 1] (centered)
  - alpha/transmittance compositing via a multiplicative scan over sorted
    gaussians; weighted color/depth reduction via Abel summation
    (sum_n w[n]*col[n] = col[0] + sum_n c[n]*(col[n+1]-col[n]))
  - separable 7x7 gaussian SSIM on the rendered window
  - L1 / SSIM / depth / opacity-entropy partial sums -> [6] outputs
"""

import os
import numpy as np

B, N, H, W = 2, 1024, 128, 128
R = 38          # window rows per core (32 owned + 3 halo each side)
RP = 40         # R padded to a multiple of 4 for the hardware loop
OWN = 32
NCORES = 8
C0 = 0.28209479177387814
C1 = 0.01 ** 2
C2 = 0.03 ** 2
EXP_N10 = float(np.exp(np.float32(-10.0)))  # exp(-10) in f32

NPIX_RGB = float(B * 3 * H * W)
NPIX_D = float(B * 1 * H * W)
NGAUSS = float(B * N)

PF_LEN = 6 * N + N + 4     # Psi + negop + c0z0, f32 words


def _ssim_g7():
    coords = np.arange(7, dtype=np.float32) - 3
    g = np.exp(-coords ** 2 / (2 * np.float32(1.5) ** 2))
    g = g / g.sum()
    return g.astype(np.float32)

G7 = _ssim_g7()


# --------------------------------------------------------------------------
# host-side per-gaussian preprocessing (numpy f32, O(N) - not the hot path)
# --------------------------------------------------------------------------

def _prep_gaussians_np(g, intr):
    """Projection + EWA 2D covariance + colors, all in f32."""
    f = np.float32
    g = g.astype(f)
    x, y, z3 = g[:, 0], g[:, 1], g[:, 2]
    s0, s1, s2 = g[:, 3], g[:, 4], g[:, 5]
    qw, qx, qy, qz = g[:, 6], g[:, 7], g[:, 8], g[:, 9]
    opac = g[:, 10]
    intr = intr.reshape(9)
    fx, cx, fy, cy = intr[0], intr[2], intr[4], intr[5]

    zcl = np.maximum(z3, f(1e-4))
    rz = f(1.0) / zcl
    px = (x * rz) * fx + cx
    py = (y * rz) * fy + cy
    zc6 = np.maximum(z3, f(1e-6))
    rzc = f(1.0) / zc6
    aJ = rzc * fx
    cJ = rzc * fy
    rzsq = rzc * rzc
    bJ = (x * rzsq) * (-fx)
    dJ = (y * rzsq) * (-fy)

    xx, yy, zz = qx * qx, qy * qy, qz * qz
    xy, xz, yz = qx * qy, qx * qz, qy * qz
    wx, wy, wz = qw * qx, qw * qy, qw * qz
    r00 = (yy + zz) * f(-2) + f(1)
    r01 = (xy - wz) * f(2)
    r02 = (xz + wy) * f(2)
    r10 = (xy + wz) * f(2)
    r11 = (xx + zz) * f(-2) + f(1)
    r12 = (yz - wx) * f(2)
    r20 = (xz - wy) * f(2)
    r21 = (yz + wx) * f(2)
    r22 = (xx + yy) * f(-2) + f(1)
    ss0, ss1, ss2 = s0 * s0, s1 * s1, s2 * s2
    t00, t01, t02 = r00 * ss0, r01 * ss1, r02 * ss2
    t10, t11, t12 = r10 * ss0, r11 * ss1, r12 * ss2
    t20, t21, t22 = r20 * ss0, r21 * ss1, r22 * ss2
    Ca = (r00 * t00 + r01 * t01) + r02 * t02
    Cb = (r00 * t10 + r01 * t11) + r02 * t12
    Cc = (r00 * t20 + r01 * t21) + r02 * t22
    Cd = (r10 * t10 + r11 * t11) + r12 * t12
    Ce = (r10 * t20 + r11 * t21) + r12 * t22
    Cf = (r20 * t20 + r21 * t21) + r22 * t22

    a2, ab, b2 = aJ * aJ, aJ * bJ, bJ * bJ
    c2, cd, d2 = cJ * cJ, cJ * dJ, dJ * dJ
    c00 = (a2 * Ca + b2 * Cf) + (ab * Cc) * f(2) + f(0.3)
    c11 = (c2 * Cd + d2 * Cf) + (cd * Ce) * f(2) + f(0.3)
    ac, ad, bc, bd = aJ * cJ, aJ * dJ, bJ * cJ, bJ * dJ
    c01 = (ac * Cb + ad * Cc) + (bc * Ce + bd * Cf)
    det = np.maximum(c00 * c11 - c01 * c01, f(1e-8))
    rdet = f(1.0) / det
    i00 = c11 * rdet
    i11 = c00 * rdet
    ni01 = c01 * rdet  # = -inv01

    pxc = px - f(64)
    pyc = py - f(64)
    psi = np.zeros((6, N), np.float32)
    psi[0] = i00 * f(-0.5)
    psi[1] = ni01
    psi[2] = i11 * f(-0.5)
    psi[3] = i00 * pxc - ni01 * pyc
    psi[4] = i11 * pyc - ni01 * pxc
    psi[5] = (pxc * psi[3] + pyc * psi[4]) * f(-0.5)

    col = np.clip(g[:, 11:14] * f(C0) + f(0.5), 0.0, 1.0).astype(f)  # [N,3]
    colz = np.concatenate([col, zcl[:, None]], axis=1)  # [N,4]
    dcolz = np.empty_like(colz)
    dcolz[:-1] = colz[1:] - colz[:-1]
    dcolz[-1] = -colz[-1]
    return psi, colz, dcolz, opac


# --------------------------------------------------------------------------
# host-side sharding
# --------------------------------------------------------------------------

def shard_inputs(gaussians, intrinsics, target_rgb, target_depth):
    gaussians = np.ascontiguousarray(gaussians, dtype=np.float32)
    intrinsics = np.ascontiguousarray(intrinsics, dtype=np.float32)
    target_rgb = np.ascontiguousarray(target_rgb, dtype=np.float32)
    target_depth = np.ascontiguousarray(target_depth, dtype=np.float32)

    z = np.maximum(gaussians[:, :, 2], 1e-4)
    order = np.argsort(z, axis=1, kind="stable")
    gs = np.take_along_axis(gaussians, order[:, :, None], axis=1)  # [B,N,38]

    # per-batch derived tensors
    per_b = []
    import ml_dtypes
    for b in range(B):
        psi, colz, dcolz, opac = _prep_gaussians_np(gs[b], intrinsics[b])
        dczT = np.ascontiguousarray(
            dcolz.reshape(8, 128, 4).transpose(1, 0, 2)
        ).astype(ml_dtypes.bfloat16)  # [128(p),8(f),4], n = f*128+p
        c0z0 = np.ascontiguousarray(colz[0].reshape(4, 1))
        negop = np.ascontiguousarray((-opac).reshape(1, N))
        # single f32 pack for the small replicated tensors (fewer host arrays)
        pack_f = np.concatenate([psi.ravel(), negop.ravel(),
                                 c0z0.ravel()]).astype(np.float32)  # [7172]
        per_b.append((pack_f, dczT))

    gx = np.arange(W, dtype=np.float32) - 64.0
    basis = np.stack([gx * gx, gx, np.ones_like(gx)])  # [3, W]

    in_maps = []
    for c in range(NCORES):
        b, q = divmod(c, 4)
        pack_f, dczT = per_b[b]
        row0 = q * OWN
        wr = np.arange(row0 - 3, row0 + OWN + 3)
        valid = (wr >= 0) & (wr < H)
        wrc = np.clip(wr, 0, H - 1)
        gyv = np.where(valid, wr.astype(np.float32) - 64.0, 0.0).astype(np.float32)

        # rank-3 factorization of Phi: phi[k,r,w] = sum_c basis[c,w]*CT[c,6r+k]
        # (rows R..RP-1 are zero padding for the hardware loop; their rendered
        # output is never read)
        CT = np.zeros((3, RP * 6), np.float32)
        for r in range(R):
            gy = gyv[r]
            CT[0, 6 * r + 0] = 1.0              # gx^2
            CT[1, 6 * r + 1] = gy               # gy*gx
            CT[2, 6 * r + 2] = gy * gy          # gy^2
            CT[1, 6 * r + 3] = 1.0              # gx
            CT[2, 6 * r + 4] = gy               # gy
            CT[2, 6 * r + 5] = 1.0              # 1
        ctb = np.ascontiguousarray(np.concatenate([CT, basis], axis=1))  # [3, 228+128]

        targ4 = np.zeros((4, R, W), np.float32)
        targ4[0:3, valid, :] = target_rgb[b][:, wrc[valid], :]
        targ4[3, valid, :] = target_depth[b, 0, wrc[valid], :]

        rowmask = np.zeros((128, 1), np.float32)
        rowmask[: 3 * R, 0] = np.tile(valid.astype(np.float32), 3)

        in_maps.append({
            "repf": pack_f.reshape(1, PF_LEN),
            "dczT": dczT,
            "ctb": ctb,
            "targ4": targ4.astype(ml_dtypes.bfloat16),
            "rowmask": rowmask,
        })
    return in_maps


def entropy_host(gaussians):
    """Opacity-entropy term: depends only on the raw inputs, so it is
    computed on the host (the sorted order does not change the sum)."""
    f = np.float32
    o = np.clip(gaussians[:, :, 10].astype(f), f(1e-6), f(1.0 - 1e-6))
    return float((o * np.log(o) + (1 - o) * np.log(1 - o)).sum(dtype=np.float64))


def combine(partials_list, ent_sum):
    S = np.zeros(5, np.float64)
    for p in partials_list:
        S += p[:5].astype(np.float64)
    l1_rgb = (S[0] + S[1] + S[2]) / NPIX_RGB
    l1_depth = S[3] / NPIX_D
    ssim = S[4] / NPIX_RGB
    ent = -ent_sum / NGAUSS
    loss = 0.8 * l1_rgb + 0.2 * (1.0 - ssim) + 0.5 * l1_depth + 0.01 * ent
    return np.float32(loss)


# --------------------------------------------------------------------------
# numpy mirror of the device program (for algorithm validation)
# --------------------------------------------------------------------------

def _conv7_np(x, axis):
    """SAME zero-padded 7-tap conv along given axis, f32, mirrors device order."""
    out = np.zeros_like(x)
    n = x.shape[axis]
    xm = np.moveaxis(x, axis, 0)
    om = np.moveaxis(out, axis, 0)
    om[:] = xm * G7[3]
    for k in [0, 1, 2, 4, 5, 6]:
        lo = max(0, 3 - k)
        hi = n + min(0, 3 - k)
        om[lo:hi] += xm[lo + k - 3: hi + k - 3] * G7[k]
    return out


def mirror_core(m, pack_f, pack_h):
    """Numpy mirror of one core's device program. Returns partials [6]."""
    f = np.float32
    psi = pack_f[0:6 * N].reshape(6, N)
    negop = pack_f[6 * N:7 * N]
    c0z0 = pack_f[7 * N:7 * N + 4]
    dcolz = pack_h.astype(f).reshape(128, 8, 4).transpose(1, 0, 2).reshape(N, 4)
    CT = m["ctb"][:, :RP * 6]
    basis = m["ctb"][:, RP * 6:]

    # render
    rend = np.zeros((4, R, W), np.float32)
    for r in range(R):
        phi_r = (CT[:, 6 * r:6 * r + 6].T @ basis).astype(f)    # [6, W]
        power = (phi_r.T.astype(f) @ psi.astype(f)).astype(f)   # [W, N]
        e = np.exp(power).astype(f)
        mneg = np.maximum(e, f(EXP_N10)) * negop[None, :]
        oma = np.maximum(mneg + f(1.0), f(0.01)).astype(f)
        c = np.cumprod(oma, axis=1, dtype=f)  # [W, N]
        acc = (c @ dcolz.astype(f)).astype(f)  # [W, 4]
        rend[0:3, r, :] = np.maximum(acc[:, 0:3] + c0z0[0:3], f(0.0)).T
        rend[3, r, :] = acc[:, 3] + c0z0[3]
    rend[0:3] = np.minimum(rend[0:3], f(1.0))

    # l1 losses (owned rows only)
    targ4 = m["targ4"].astype(f)
    omask = np.zeros((4, R, W), np.float32)
    omask[:, 3:3 + OWN, :] = 1.0
    l1d = np.abs(rend - targ4).astype(f)
    lacc = (l1d * omask).reshape(4, -1).sum(axis=1, dtype=f)

    # ssim on the window
    rowmask = m["rowmask"][: 3 * R, 0].reshape(3, R)
    img1 = rend[0:3] * rowmask[:, :, None]
    img2 = targ4[0:3]
    i11 = img1 * img1
    i22 = img2 * img2
    i12 = img1 * img2
    outs = []
    for xin in (img1, img2, i11, i22, i12):
        rc = _conv7_np(xin.astype(f), axis=2)     # along W
        hc = _conv7_np(rc.astype(f), axis=1)      # along rows (full window)
        outs.append(hc[:, 3:3 + OWN, :].astype(f))
    mu1, mu2, M11, M22, M12 = outs
    A = mu1 * mu2
    num = (A * f(2) + f(C1)) * ((M12 - A) * f(2) + f(C2))
    Cq = mu1 * mu1
    Dq = mu2 * mu2
    den = ((Cq + f(C1)) + Dq) * (((M11 - Cq) + f(C2)) + (M22 - Dq))
    smap = (num / den).astype(f)
    ssum = smap.sum(dtype=f)

    return np.array([lacc[0], lacc[1], lacc[2], lacc[3], ssum, 0.0], np.float32)


def kernel_numpy(**inputs):
    """Full numpy mirror (no device) - for validation."""
    in_maps = shard_inputs(**inputs)
    partials = [mirror_core(m, m["repf"][0], m["dczT"].ravel())
                for m in in_maps]
    return combine(partials, entropy_host(np.asarray(inputs["gaussians"],
                                                     dtype=np.float32)))


# --------------------------------------------------------------------------
# device program
# --------------------------------------------------------------------------

_PROG_CACHE = {}


def build_program(debug_rend=False):
    import concourse.bass as bass
    import concourse.bacc as bacc
    import concourse.tile as tile
    import concourse.mybir as mybir
    from concourse.masks import make_identity

    F32 = mybir.dt.float32
    BF16 = mybir.dt.bfloat16
    OP = mybir.AluOpType
    ACT = mybir.ActivationFunctionType

    nc = bacc.Bacc("TRN2", target_bir_lowering=False, debug=False,
                   num_devices=NCORES)
    repf_in = nc.dram_tensor("repf", [1, PF_LEN], F32, kind="ExternalInput").ap()
    dczT_in = nc.dram_tensor("dczT", [128, 8, 4], BF16, kind="ExternalInput").ap()
    ctb_in = nc.dram_tensor("ctb", [3, RP * 6 + W], F32, kind="ExternalInput").ap()
    targ4_in = nc.dram_tensor("targ4", [4, R, W], BF16, kind="ExternalInput").ap()
    rowmask_in = nc.dram_tensor("rowmask", [128, 1], F32, kind="ExternalInput").ap()
    partials = nc.dram_tensor("partials", [6], F32, kind="ExternalOutput").ap()
    if debug_rend:
        dbg_rend = nc.dram_tensor("dbg_rend", [4, R, W], F32, kind="ExternalOutput").ap()

    V = nc.vector
    S = nc.scalar
    T = nc.tensor
    G = nc.gpsimd

    with tile.TileContext(nc) as tc:
        with (
            tc.tile_pool(name="const", bufs=1) as cp,
            tc.tile_pool(name="loop", bufs=1) as lp,
            tc.tile_pool(name="ppow", bufs=1, space="PSUM") as ppow,
            tc.tile_pool(name="pmisc", bufs=1, space="PSUM") as pmisc,
            tc.tile_pool(name="dram", bufs=1, space="DRAM") as dp,
        ):
            # ---------------- constants / loads ----------------
            idt = cp.tile([128, 128], F32, tag="identity", name="identity")
            make_identity(nc, idt[:])
            ones_col = cp.tile([128, 1], F32, tag="ones_col", name="ones_col")
            G.memset(ones_col[:], 1.0)
            ones_row = cp.tile([1, 128], F32, tag="ones_row", name="ones_row")
            G.memset(ones_row[:], 1.0)

            Psi = cp.tile([6, N], F32, tag="Psi", name="Psi")
            nc.sync.dma_start(Psi[:],
                              repf_in[0, 0:6 * N].rearrange("(k n) -> k n", k=6))
            nrow = cp.tile([1, N], F32, tag="nrow", name="nrow")
            nc.sync.dma_start(nrow[:],
                              repf_in[0, 6 * N:7 * N].rearrange("(a n) -> a n", a=1))
            c0z0 = cp.tile([4, 1], F32, tag="c0z0", name="c0z0")
            nc.sync.dma_start(c0z0[:],
                              repf_in[0, 7 * N:7 * N + 4].rearrange("(a b) -> a b", a=4))
            dcz = cp.tile([128, 8, 4], BF16, tag="dcz", name="dcz")
            nc.sync.dma_start(dcz[:], dczT_in[:])
            ctb = cp.tile([3, RP * 6 + W], F32, tag="ctb", name="ctb")
            nc.sync.dma_start(ctb[:], ctb_in[:])

            targ4_bf = cp.tile([4, R, W], BF16, tag="targ4_bf", name="targ4_bf")
            nc.sync.dma_start(targ4_bf[:], targ4_in[:])
            targ4_sb = cp.tile([4, R, W], F32, tag="targ4_sb", name="targ4_sb")
            V.tensor_copy(targ4_sb[:], targ4_bf[:])
            targc_bf = cp.tile([128, W], BF16, tag="targc_bf", name="targc_bf")
            G.memset(targc_bf[:], 0.0)
            nc.sync.dma_start(targc_bf[0:114, :],
                              targ4_in[0:3].rearrange("c r w -> (c r) w"))
            targc = cp.tile([128, W], F32, tag="targc", name="targc")
            V.tensor_copy(targc[:], targc_bf[:])
            rowm = cp.tile([128, 1], F32, tag="rowm", name="rowm")
            nc.sync.dma_start(rowm[:], rowmask_in[:])

            omask = cp.tile([4, R, W], F32, tag="omask", name="omask")
            G.memset(omask[:], 0.0)
            G.memset(omask[:, 3:3 + OWN, :], 1.0)

            # ---------------- negop broadcast [128, N] ----------------
            negopb = cp.tile([128, N], F32, tag="negopb", name="negopb")
            nps = ppow.tile([128, N], F32, tag="pow0", name="nps")
            T.matmul(nps[:, 0:512], ones_row[:], nrow[:, 0:512], start=True, stop=True)
            T.matmul(nps[:, 512:1024], ones_row[:], nrow[:, 512:1024], start=True, stop=True)
            V.tensor_copy(negopb[:], nps[:])

            # ---------------- Phi reconstruction ----------------
            # phiT[w, 6r+k] = sum_c basis[c,w] * CT[c,6r+k]; PE-transpose to
            # [(6r+k), w] chunks, bounce through DRAM; the render loop stages
            # each row's [6, W] slice back by DMA (PE lhsT needs a static
            # address, and base partition must be 0/32/64).
            KR = RP * 6  # 240
            bphi = pmisc.tile([128, KR], F32, tag="tp", name="bphi")
            T.matmul(bphi[:], ctb[:, KR:KR + W], ctb[:, 0:KR], start=True, stop=True)
            phiT_sb = cp.tile([128, KR], F32, tag="phiT_sb", name="phiT_sb")
            V.tensor_copy(phiT_sb[:], bphi[:])
            phiP = cp.tile([128, 2, 128], F32, tag="phiP", name="phiP")
            trA = pmisc.tile([128, 128], F32, tag="tp", name="trA")
            T.transpose(trA[0:126, :], phiT_sb[:, 0:126], idt[:])
            V.tensor_copy(phiP[0:126, 0, :], trA[0:126, :])
            trB = pmisc.tile([128, 128], F32, tag="tp", name="trB")
            T.transpose(trB[0:114, :], phiT_sb[:, 126:240], idt[:])
            V.tensor_copy(phiP[0:114, 1, :], trB[0:114, :])
            phi_scr = dp.tile([KR, W], F32, tag="phi_scr", name="phi_scr")
            nc.sync.dma_start(phi_scr[0:126, :], phiP[0:126, 0, :])
            nc.sync.dma_start(phi_scr[126:240, :], phiP[0:114, 1, :])

            # ---------------- render loop (hardware loop, 4 rows/iter) ------
            rend_all = cp.tile([4, RP, W], F32, tag="rend_all", name="rend_all")
            NK = N // 128
            rend_flat = rend_all[:].rearrange("c r w -> c (r w)")

            # statically allocated per-slot tiles (manual double buffer)
            pw_t = [ppow.tile([128, N], F32, tag=f"pow{j}", name=f"pow{j}")
                    for j in range(2)]
            er_t = [lp.tile([128, N], F32, tag=f"eraw{j}", name=f"eraw{j}")
                    for j in range(2)]
            om_t = [lp.tile([128, N], F32, tag=f"oma{j}", name=f"oma{j}")
                    for j in range(2)]
            ct_t = [lp.tile([128, N], BF16, tag=f"ctile{j}", name=f"ctile{j}")
                    for j in range(2)]
            csb_t = [lp.tile([128, NK, 128], BF16, tag=f"ctsb{j}", name=f"ctsb{j}")
                     for j in range(2)]
            phi_t = [lp.tile([6, W], F32, tag=f"phis{j}", name=f"phis{j}")
                     for j in range(2)]
            acc2 = pmisc.tile([4, 2, 128], F32, tag="acc", name="acc")

            from concourse.bass import ds

            def render_row(row_i, j):
                """row_i: ScalarValue row index; j: buffer slot (0/1)."""
                pw, er, om, ct = pw_t[j], er_t[j], om_t[j], ct_t[j]
                csb, acc = csb_t[j], acc2[:, j, :]
                phi_r = phi_t[j]
                nc.sync.dma_start(phi_r[:], phi_scr[ds(row_i * 6, 6), :])
                T.matmul(pw[:, 0:512], phi_r[:], Psi[:, 0:512],
                         start=True, stop=True)
                T.matmul(pw[:, 512:1024], phi_r[:], Psi[:, 512:1024],
                         start=True, stop=True)
                S.activation(er[:], pw[:], ACT.Exp, bias=0.0, scale=1.0)
                V.scalar_tensor_tensor(om[:], er[:], EXP_N10, negopb[:],
                                       OP.max, OP.mult)
                G.tensor_scalar(om[:], om[:], 1.0, 0.01, OP.add, OP.max)
                V.tensor_tensor_scan(ct[:], om[:], om[:], 1.0, OP.mult, OP.bypass)
                nc.sync.dma_start_transpose(csb[:], ct[:])
                for k in range(NK):
                    T.matmul(acc, dcz[:, k, :], csb[:, k, :],
                             start=(k == 0), stop=(k == NK - 1))
                V.tensor_scalar(rend_flat[:, ds(row_i * W, W)], acc,
                                c0z0[:, :], 0.0, OP.add, OP.max)

            with tc.For_i(0, RP, 4) as rv:
                for jj in range(4):
                    render_row(rv + jj, jj % 2)

            V.tensor_scalar(rend_all[0:3], rend_all[0:3], 1.0, None,
                            OP.min, OP.bypass)

            if debug_rend:
                nc.sync.dma_start(dbg_rend[:], rend_all[:, 0:R, :])

            # ---------------- L1 losses ----------------
            l1d = cp.tile([4, R, W], F32, tag="l1d", name="l1d")
            V.tensor_sub(l1d[:], rend_all[:, 0:R, :], targ4_sb[:])
            S.activation(l1d[:], l1d[:], ACT.Abs, bias=0.0, scale=1.0)
            lacc = cp.tile([4, 1], F32, tag="lacc", name="lacc")
            V.tensor_mul(l1d[:], l1d[:], omask[:])
            V.tensor_reduce(lacc[:], l1d[:], axis=mybir.AxisListType.XY, op=OP.add)

            # ---------------- SSIM ----------------
            img1 = cp.tile([128, W], F32, tag="img1", name="img1")
            G.memset(img1[:], 0.0)
            for ch in range(3):
                nc.sync.dma_start(img1[ch * R:(ch + 1) * R, :],
                                  rend_all[ch:ch + 1, 0:R, :])
            V.tensor_scalar(img1[:], img1[:], rowm[:], None,
                            OP.mult, OP.bypass)
            i11t = cp.tile([128, W], F32, tag="i11t", name="i11t")
            V.tensor_mul(i11t[:], img1[:], img1[:])
            i22t = cp.tile([128, W], F32, tag="i22t", name="i22t")
            V.tensor_mul(i22t[:], targc[:], targc[:])
            i12t = cp.tile([128, W], F32, tag="i12t", name="i12t")
            V.tensor_mul(i12t[:], img1[:], targc[:])

            # separable 7-tap convs as banded-matrix matmuls.
            # Kw[p,f] = g7[f-p+3] (full-width band); Kb = same band restricted
            # to the three 38-row channel blocks (rows 0:114).
            g7 = [float(v) for v in G7]
            Kw = cp.tile([128, 128], F32, tag="Kw", name="Kw")
            G.memset(Kw[:], 0.0)
            for d in range(-3, 4):
                G.affine_select(out=Kw[:], in_=Kw[:],
                                compare_op=OP.not_equal, fill=g7[d + 3],
                                base=-d, pattern=[[1, 128]],
                                channel_multiplier=-1)
            # cross-channel leakage from the full-width row band only lands in
            # halo output rows (discarded by the 3:35 owned slice), and input
            # rows >= 114 are zero, so the same Kw serves both conv directions.
            convs = []
            for j, xin in enumerate([img1, targc, i11t, i22t, i12t]):
                p1 = pmisc.tile([128, 128], F32, tag="tp", name=f"cv{j}a")
                T.matmul(p1[0:114, :], Kw[:, 0:114], xin[:], start=True, stop=True)
                s1 = cp.tile([128, W], F32, tag="ssrc", name=f"rc{j}")
                V.tensor_copy(s1[0:114, :], p1[0:114, :])
                p2 = pmisc.tile([128, 128], F32, tag="tp", name=f"cv{j}b")
                T.transpose(p2[:, 0:114], s1[0:114, :], idt[0:114, 0:114])
                s2 = cp.tile([128, 128], F32, tag="ssrcT", name=f"rcT{j}")
                V.tensor_copy(s2[:, 0:114], p2[:, 0:114])
                s2v = s2[:, 0:114].rearrange("p (c r) -> p c r", c=3)
                p3 = pmisc.tile([128, 3, OWN], F32, tag="tp2", name=f"cv{j}c")
                T.matmul(p3[:], Kw[:], s2v[:, :, 3:3 + OWN], start=True, stop=True)
                mu = cp.tile([128, 3, OWN], F32, tag=f"mu{j}", name=f"mu{j}")
                V.tensor_copy(mu[:], p3[:])
                convs.append(mu)
            mu1, mu2, M11, M22, M12 = convs

            def big(tag, name):
                return cp.tile([128, 3, OWN], F32, tag=tag, name=name)

            t1 = big("sst1", "ssA")       # A = mu1*mu2
            V.tensor_mul(t1[:], mu1[:], mu2[:])
            t2 = big("sst2", "ssnum1")    # num1 = 2A + C1
            V.tensor_scalar(t2[:], t1[:], 2.0, C1, OP.mult, OP.add)
            t3 = big("sst3", "ssB")       # B = M12 - A; num2 = 2B + C2
            V.tensor_sub(t3[:], M12[:], t1[:])
            V.tensor_scalar(t3[:], t3[:], 2.0, C2, OP.mult, OP.add)
            V.tensor_mul(t2[:], t2[:], t3[:])            # num
            V.tensor_mul(t1[:], mu1[:], mu1[:])          # Cq
            V.tensor_mul(t3[:], mu2[:], mu2[:])          # Dq
            t4 = big("sst4", "ssden1")    # den1 = Cq + C1 + Dq
            V.scalar_tensor_tensor(t4[:], t1[:], C1, t3[:], OP.add, OP.add)
            V.tensor_sub(M11[:], M11[:], t1[:])          # E = M11 - Cq
            V.tensor_sub(M22[:], M22[:], t3[:])          # F = M22 - Dq
            V.scalar_tensor_tensor(M11[:], M11[:], C2, M22[:],
                                   OP.add, OP.add)       # den2
            V.tensor_mul(t4[:], t4[:], M11[:])           # den
            V.reciprocal(t4[:], t4[:])
            V.tensor_mul(t2[:], t2[:], t4[:])            # smap
            ssum = cp.tile([128, 1], F32, tag="ssum", name="ssum")
            V.tensor_reduce(ssum[:], t2[:], axis=mybir.AxisListType.XY, op=OP.add)
            sp = pmisc.tile([1, 1], F32, tag="tp", name="tp2")
            T.matmul(sp[:], ssum[:], ones_col[:], start=True, stop=True)

            # ---------------- outputs ----------------
            outsb = cp.tile([1, 1], F32, tag="outsb", name="outsb")
            V.tensor_copy(outsb[:, 0:1], sp[:])
            nc.sync.dma_start(partials[0:4], lacc[:, 0])
            nc.sync.dma_start(partials[4:5], outsb[0, :])

    nc.compile()
    return nc


def _get_program(debug_rend=False):
    key = ("prog", debug_rend)
    if key not in _PROG_CACHE:
        _PROG_CACHE[key] = build_program(debug_rend)
    return _PROG_CACHE[key]


def run_device(in_maps, mode="hw", debug_rend=False):
    nc = _get_program(debug_rend)
    if mode == "sim":
        from concourse.bass_interp import MultiCoreSim
        sim = MultiCoreSim(nc, num_cores=len(in_maps))
        for i, m in enumerate(in_maps):
            for k, v in m.items():
                sim.cores[i].tensor(k)[:] = v
        sim.simulate(check_with_hw=False)
        names = ["partials"] + (["dbg_rend"] if debug_rend else [])
        return [{n: np.array(sim.cores[i].tensor(n)) for n in names}
                for i in range(len(in_maps))]
    from concourse.bass_utils import run_bass_kernel_spmd
    res = run_bass_kernel_spmd(nc, in_maps, list(range(len(in_maps))))
    return res.results


def kernel(**inputs):
    in_maps = shard_inputs(**inputs)
    mode = os.environ.get("GK_MODE", "hw")
    results = run_device(in_maps, mode=mode)
    ent = entropy_host(np.asarray(inputs["gaussians"], dtype=np.float32))
    return combine([r["partials"] for r in results], ent)


if __name__ == "__main__":
    import jax
    with jax.default_device(jax.devices("cpu")[0]):
        import reference
        inputs = {k: np.asarray(v) for k, v in reference.setup_inputs().items()}
        expected = float(reference.reference(**inputs))
    got = float(kernel_numpy(**inputs))
    rel = abs(got - expected) / max(abs(expected), 1e-12)
    print(f"expected {expected:.8f}  mirror {got:.8f}  rel {rel:.3e}")
